# revision 36
# baseline (speedup 1.0000x reference)
"""Trainium2 Bass kernel for nn_MapLoss (topk_masking).

Strategy
--------
The reference loss needs, per sample and per map (region / affinity), only
three reductions:

    S_tot = sum(d^2 * mask)                 (d = clamped pred - gt)
    S_pos = sum((gt > t) * d^2 * mask)
    n_pos = #(gt > t)

because for the hard-negative top-k, k = min(3*n_pos, n_neg) and whenever
k == n_neg the "top-k sum of negatives" is just S_tot - S_pos (sum of all
negatives).  The rare general branches (3*n_pos < n_neg, or n_pos == 0)
are handled by an exact host fallback per sample (statistically never
taken for this input distribution).

Device plan (pure data parallel, 4 samples per core, no collectives):
  per sample-map tile [128, 2048] f32:
    1. custom DVE op CLAMPED_DIFF : d = (pred-gt) - (gt>t)*relu(pred-1)
    2. custom DVE op MASKED_SQ    : l = d^2*mask,  accum-> S_tot   (fused)
    3. scalar_tensor_tensor       : (gt>t)*l,      accum-> S_pos   (fused)
    4. ScalarE Sign activation    : sign(gt-t-eps), accum-> 2*n_pos - N
  Per-core output: per-partition stats tiles, final reduction on host.
"""

import os
import numpy as np
from contextlib import ExitStack

from concourse import bass, bacc, mybir
from concourse import tile
from concourse import bass_utils
import concourse.dve_ops as dve_ops_mod
from concourse.dve_ops import DveOp
from concourse.dve_spec import (
    Spec,
    Src0,
    Src1,
    C0,
    C1,
    Zero,
    relu,
    sq,
    lower,
    _has_src1,
)
from concourse.dve_uop import DveOpSpec
from operator import add as _op_add

# ---------------------------------------------------------------- constants
_B, _H, _W = 32, 512, 512
_N = _H * _W            # 262144 elements / sample
_P, _F = 128, 2048      # on-chip tile: 128 partitions x 2048 free  (= _N)
_NCORES = 8
_SPC = _B // _NCORES    # 4 samples per core
_T_G = 0.6              # THRESH_POSITIVE_REGION
_T_A = 0.65             # THRESH_POSITIVE_AFFINITY
_LAMBDA = 2.0
_TOPK_FALLBACK = 500

_IN_NAMES = ("rgt", "agt", "rpred", "apred", "m")

# ------------------------------------------------- custom DVE op definitions


def _register_dve_op(name, spec, subdim=False):
    """Register a custom DVE op in the process-local registry (additive;
    the documented extension point is appending to dve_ops.OPS)."""
    if name in dve_ops_mod._SUB_OPCODE_FOR_NAME:
        for op in dve_ops_mod.OPS:
            if op.name == name:
                return op
        raise RuntimeError(f"{name} in opcode map but not in OPS")
    row = max(dve_ops_mod._SUB_OPCODE_FOR_NAME.values()) + 1
    assert row < 0x20, "custom DVE opcode rows exhausted"
    shas = {}
    for ver in ("v3", "v4"):
        try:
            tmp = DveOpSpec(
                name=name, opcode=row, uops=lower(spec, ver=ver),
                rd1_en=_has_src1(spec),
            )
            shas[ver] = tmp.sha(ver)
        except Exception:
            pass
    assert "v3" in shas, f"{name}: failed to lower for TRN2"
    op = DveOp(name, spec, subdim, uops_sha=shas)
    dve_ops_mod.OPS.append(op)
    dve_ops_mod._SUB_OPCODE_FOR_NAME[name] = row
    dve_ops_mod.CUSTOM_DVE_SPECS[name] = spec
    return op


_OPS_CACHE = {}


def _get_custom_ops():
    if _OPS_CACHE:
        return _OPS_CACHE

    # d = (pred - gt) - (gt > t) * relu(pred - 1)
    clamped_diff = Spec(
        body=(Src0 - Src1) - (Src1 > C0) * relu(Src0 - C1),
        reference=lambda in0, in1, s0, s1, imm2: (
            (in0.astype(np.float32) - in1)
            - (in1 > s0) * np.maximum(in0.astype(np.float32) - s1, 0.0)
        ).astype(np.float32),
    )

    # l = in0^2 * in1 ; accum_out = sum(l)
    def _masked_sq_ref(in0, in1, s0, s1, imm2):
        b = (np.square(in0.astype(np.float32)) * in1).astype(np.float32)
        return b, b.reshape(b.shape[0], -1).sum(axis=-1, keepdims=True)

    masked_sq = Spec(
        body=sq(Src0) * Src1,
        accum=_op_add,
        accum_init=Zero,
        reference=_masked_sq_ref,
    )

    _OPS_CACHE["clamped_diff"] = _register_dve_op("ANT_MAPLOSS_CLAMPED_DIFF", clamped_diff)
    _OPS_CACHE["masked_sq"] = _register_dve_op("ANT_MAPLOSS_MASKED_SQ", masked_sq)
    return _OPS_CACHE


# ------------------------------------------------------------- bass builder

_NC_CACHE = {}


def _f32_exact(x):
    return float(np.float32(x))


def _build_bass(use_custom=True, repeats=1, loop=False, n_pos_mode="act_sign",
                dma="sync", frame=1, bufs_in=2, bufs_work=2):
    key = ("nc", use_custom, repeats, loop, n_pos_mode, dma, frame, bufs_in,
           bufs_work)
    if key in _NC_CACHE:
        return _NC_CACHE[key]
    ops = _get_custom_ops() if use_custom else None
    assert _SPC % frame == 0

    f32 = mybir.dt.float32
    bf16 = mybir.dt.bfloat16

    nc = bacc.Bacc(
        "TRN2", target_bir_lowering=False, debug=False, num_devices=_NCORES
    )
    ins = {
        name: nc.dram_tensor(
            name, [_SPC, _P, _F], f32, kind="ExternalInput"
        ).ap()
        for name in _IN_NAMES
    }
    # per sample-map stats: (S_tot, S_pos, n_pos) x 4 samples x 2 maps
    stats_v = nc.dram_tensor("stats_v", [_P, 24], f32, kind="ExternalOutput").ap()
    # scalar-engine stats (sign-sums = 2*n_pos - N), separate tile to avoid
    # cross-engine false deps on sv
    stats_s = nc.dram_tensor("stats_s", [_P, 8], f32, kind="ExternalOutput").ap()

    with tile.TileContext(nc) as tc, ExitStack() as ctx:
        inpool = ctx.enter_context(tc.tile_pool(name="in", bufs=bufs_in))
        workpool = ctx.enter_context(tc.tile_pool(name="work", bufs=bufs_work))
        statpool = ctx.enter_context(tc.tile_pool(name="stat", bufs=1))

        dma_eng = nc.sync if dma == "sync" else nc.gpsimd

        sv = statpool.tile([_P, 24], f32)
        ss = statpool.tile([_P, 8], f32)
        nc.scalar.memzero(ss[:])

        sign_bias = {}
        if n_pos_mode == "act_sign":
            # eps = 2^-24 shifts the threshold off the 2^-23 input grid so
            # sign() reproduces the strict '>' exactly (never hits 0).
            for thr in (_T_G, _T_A):
                bt = statpool.tile([_P, 1], f32, tag=f"bias{int(thr * 100)}")
                nc.gpsimd.memset(bt[:], -(_f32_exact(thr) + 2.0 ** -24))
                sign_bias[thr] = bt
        relu_bias = None
        if not use_custom:
            relu_bias = statpool.tile([_P, 1], f32, tag="relu_bias")
            nc.gpsimd.memset(relu_bias[:], -1.0)

        def emit_frame(f0):
            tiles = {}
            for name in _IN_NAMES:
                t = inpool.tile([_P, frame, _F], f32, tag=name)
                dma_eng.dma_start(
                    out=t[:],
                    in_=ins[name][f0 : f0 + frame].rearrange("s p f -> p s f"),
                )
                tiles[name] = t

            for j in range(frame):
                s = f0 + j
                for mi, (gtn, prn, thr) in enumerate(
                    (("rgt", "rpred", _T_G), ("agt", "apred", _T_A))
                ):
                    gt_t = tiles[gtn][:, j]
                    pr_t = tiles[prn][:, j]
                    m_t = tiles["m"][:, j]
                    thr32 = _f32_exact(thr)
                    col = (s * 2 + mi) * 3    # S_tot / S_pos / n_pos columns

                    d = workpool.tile([_P, _F], f32, tag="d")
                    if use_custom:
                        nc.vector._custom_dve(
                            ops["clamped_diff"], out=d[:], in0=pr_t,
                            in1=gt_t, s0=thr32, s1=1.0,
                        )
                    else:
                        # fallback chain built only from scalar_tensor_tensor
                        # and activation forms (same instruction structs as
                        # the custom path uses):
                        #   r = relu(pred-1); w = (gt>t)*r
                        #   t1 = (w*-1) + pred; d = (gt*-1) + t1
                        r = workpool.tile([_P, _F], f32, tag="r")
                        nc.scalar.activation(
                            r[:], pr_t, mybir.ActivationFunctionType.Relu,
                            bias=relu_bias[:], scale=1.0,
                        )
                        w = workpool.tile([_P, _F], f32, tag="w")
                        nc.vector.scalar_tensor_tensor(
                            out=w[:], in0=gt_t, scalar=thr32, in1=r[:],
                            op0=mybir.AluOpType.is_gt,
                            op1=mybir.AluOpType.mult,
                        )
                        t1 = workpool.tile([_P, _F], f32, tag="t1")
                        nc.vector.scalar_tensor_tensor(
                            out=t1[:], in0=w[:], scalar=-1.0, in1=pr_t,
                            op0=mybir.AluOpType.mult,
                            op1=mybir.AluOpType.add,
                        )
                        nc.vector.scalar_tensor_tensor(
                            out=d[:], in0=gt_t, scalar=-1.0, in1=t1[:],
                            op0=mybir.AluOpType.mult,
                            op1=mybir.AluOpType.add,
                        )

                    l = workpool.tile([_P, _F], f32, tag="l")
                    if use_custom:
                        nc.vector._custom_dve(
                            ops["masked_sq"], out=l[:], in0=d[:], in1=m_t,
                            accum_out=sv[:, col : col + 1],
                        )
                    else:
                        # dm = d*m ; l = d*dm with fused accum -> S_tot
                        dm = workpool.tile([_P, _F], f32, tag="dm")
                        nc.vector.scalar_tensor_tensor(
                            out=dm[:], in0=d[:], scalar=0.0, in1=m_t,
                            op0=mybir.AluOpType.bypass,
                            op1=mybir.AluOpType.mult,
                        )
                        nc.vector.scalar_tensor_tensor(
                            out=l[:], in0=d[:], scalar=0.0, in1=dm[:],
                            op0=mybir.AluOpType.bypass,
                            op1=mybir.AluOpType.mult,
                            accum_out=sv[:, col : col + 1],
                        )

                    z = workpool.tile([_P, 1], f32, tag="z")
                    nc.vector.scalar_tensor_tensor(
                        out=z.broadcast_to((_P, _F)), in0=gt_t, scalar=thr32,
                        in1=l[:],
                        op0=mybir.AluOpType.is_gt, op1=mybir.AluOpType.mult,
                        accum_out=sv[:, col + 1 : col + 2],
                    )

                    if n_pos_mode == "act_sign":
                        zs = workpool.tile([_P, _F], bf16, tag="zs")
                        nc.scalar.activation(
                            zs[:], gt_t, mybir.ActivationFunctionType.Sign,
                            bias=sign_bias[thr][:], scale=1.0,
                            accum_out=ss[:, s * 2 + mi : s * 2 + mi + 1],
                        )
                    else:
                        zp = workpool.tile([_P, 1], f32, tag="zp")
                        nc.vector.tensor_scalar(
                            out=zp.broadcast_to((_P, _F)), in0=gt_t,
                            scalar1=thr32,
                            scalar2=None, op0=mybir.AluOpType.is_gt,
                            op1=mybir.AluOpType.add,
                            accum_out=sv[:, col + 2 : col + 3],
                        )

        def emit_body():
            for f0 in range(0, _SPC, frame):
                emit_frame(f0)

        if loop and repeats > 1:
            with tc.For_i(0, repeats, 1):
                emit_body()
        else:
            for _ in range(repeats):
                emit_body()

        dma_eng.dma_start(out=stats_v[:], in_=sv[:])
        dma_eng.dma_start(out=stats_s[:], in_=ss[:])

    nc.compile()
    _NC_CACHE[key] = nc
    return nc


# ------------------------------------------------------------ host fallback


def _host_sample_loss(pre_loss, label, thresh):
    """Exact per-sample replica of reference._single_image_loss (one sample)."""
    pre_loss = pre_loss.astype(np.float64).ravel()
    label = label.astype(np.float32).ravel()
    pos_mask = label > np.float32(thresh)
    n_pos = int(pos_mask.sum())
    n_neg = pre_loss.size - n_pos
    if n_pos == 0:
        top = np.sort(pre_loss)[::-1][:_TOPK_FALLBACK]
        return float(top.mean())
    pos_loss = pre_loss[pos_mask].sum() / n_pos
    k = min(3 * n_pos, n_neg)
    if k <= 0:
        return float(pos_loss)
    neg_vals = np.sort(pre_loss[~pos_mask])[::-1]
    neg_loss = neg_vals[:k].sum() / k
    return float(pos_loss + neg_loss)


def _host_pre_loss(gt, pred, mask, thresh):
    gt = gt.astype(np.float32)
    pred = pred.astype(np.float32)
    clamped = np.where((gt > np.float32(thresh)) & (pred > np.float32(1.0)),
                       np.float32(1.0), pred)
    d = clamped.astype(np.float64) - gt.astype(np.float64)
    return d * d * mask.astype(np.float64)


# ------------------------------------------------------------------- bench


def _io_spec(nc):
    """Mirror run_bass_via_pjrt's input/output discovery."""
    partition_name = (
        nc.partition_id_tensor.name if nc.partition_id_tensor else None
    )
    in_names, out_names, out_avals, zero_outs = [], [], [], []
    import jax

    for alloc in nc.m.functions[0].allocations:
        if not isinstance(alloc, mybir.MemoryLocationSet):
            continue
        name = alloc.memorylocations[0].name
        if alloc.kind == "ExternalInput":
            if name != partition_name:
                in_names.append(name)
        elif alloc.kind == "ExternalOutput":
            out_names.append(name)
            shape = tuple(alloc.tensor_shape)
            dtype = mybir.dt.np(alloc.dtype)
            out_avals.append(jax.core.ShapedArray(shape, dtype))
            zero_outs.append(np.zeros(shape, dtype))
    return partition_name, in_names, out_names, out_avals, zero_outs


def _bench_one(inputs, iters=30, warmup=2, **build_kw):
    """Amortized per-execution wall time (ns) over `iters` queued runs."""
    import time
    import jax
    from jax.sharding import Mesh, PartitionSpec
    from jax.experimental.shard_map import shard_map
    from concourse import bass2jax
    from concourse.bass2jax import _bass_exec_p, install_neuronx_cc_hook

    install_neuronx_cc_hook()
    nc = _build_bass(**build_kw)
    pname, in_names, out_names, out_avals, zero_outs = _io_spec(nc)
    n_params, n_outs = len(in_names), len(out_names)
    all_names = in_names + out_names + ([pname] if pname else [])

    def _body(*args):
        operands = list(args)
        if pname is not None:
            operands.append(bass2jax.partition_id_tensor())
        outs = _bass_exec_p.bind(
            *operands,
            out_avals=tuple(out_avals),
            in_names=tuple(all_names),
            out_names=tuple(out_names),
            lowering_input_output_aliases=(),
            sim_require_finite=True,
            sim_require_nnan=True,
            nc=nc,
        )
        return tuple(outs)

    devices = jax.devices()[:_NCORES]
    mesh = Mesh(np.asarray(devices), ("core",))
    in_specs = (PartitionSpec("core"),) * (n_params + n_outs)
    out_specs = (PartitionSpec("core"),) * n_outs
    donate = tuple(range(n_params, n_params + n_outs))
    sharded = jax.jit(
        shard_map(_body, mesh=mesh, in_specs=in_specs, out_specs=out_specs,
                  check_rep=False),
        donate_argnums=donate, keep_unused=True,
    )

    arr = {k: np.ascontiguousarray(np.asarray(v, np.float32).reshape(_B, _P, _F))
           for k, v in inputs.items()}
    keyed = {"rgt": arr["region_score_gt"], "agt": arr["affinity_score_gt"],
             "rpred": arr["region_score_pred"], "apred": arr["affinity_score_pred"],
             "m": arr["mask"]}
    concat_in = [keyed[name] for name in in_names]  # [32,128,2048] = 8 cores x 4
    dev_in = [jax.device_put(a) for a in concat_in]

    def zeros():
        return [np.zeros((_NCORES * z.shape[0], *z.shape[1:]), z.dtype)
                for z in zero_outs]

    for _ in range(warmup):
        outs = sharded(*dev_in, *zeros())
        jax.block_until_ready(outs)
    zs = [zeros() for _ in range(iters)]
    t0 = time.perf_counter()
    results = [sharded(*dev_in, *z) for z in zs]
    jax.block_until_ready(results)
    t1 = time.perf_counter()
    return (t1 - t0) / iters * 1e9


def bench(inputs, rounds=3, k_lo=400, k_hi=1200, **build_kw):
    """Device time per kernel body (ns): slope between on-device For_i loops
    of k_lo and k_hi iterations.  K must be large enough that device time
    dominates the dispatch roundtrip, else async dispatch hides it."""
    est = []
    for _ in range(rounds):
        lo = _bench_one(inputs, iters=4, repeats=k_lo, loop=True, **build_kw)
        hi = _bench_one(inputs, iters=4, repeats=k_hi, loop=True, **build_kw)
        est.append((hi - lo) / (k_hi - k_lo))
    return float(np.median(est))


# ------------------------------------------------------------------- kernel

LAST_RESULTS = None


def kernel(**inputs):
    global LAST_RESULTS
    arr = {
        k: np.ascontiguousarray(np.asarray(v, dtype=np.float32))
        for k, v in inputs.items()
    }
    rgt = arr["region_score_gt"].reshape(_B, _P, _F)
    agt = arr["affinity_score_gt"].reshape(_B, _P, _F)
    rpred = arr["region_score_pred"].reshape(_B, _P, _F)
    apred = arr["affinity_score_pred"].reshape(_B, _P, _F)
    m = arr["mask"].reshape(_B, _P, _F)

    use_custom = os.environ.get("MAPLOSS_NO_CUSTOM", "0") != "1"
    n_pos_mode = os.environ.get("MAPLOSS_NPOS", "act_sign")
    dma = os.environ.get("MAPLOSS_DMA", "sync")
    nc = _build_bass(use_custom=use_custom, n_pos_mode=n_pos_mode, dma=dma)

    in_maps = []
    for c in range(_NCORES):
        sl = slice(c * _SPC, (c + 1) * _SPC)
        in_maps.append(
            {"rgt": rgt[sl], "agt": agt[sl], "rpred": rpred[sl],
             "apred": apred[sl], "m": m[sl]}
        )

    res = bass_utils.run_bass_kernel_spmd(
        nc, in_maps, core_ids=list(range(_NCORES))
    )
    LAST_RESULTS = res

    # ---- host-side finish (tiny): per-sample scalars ----------------------
    per_sample = np.zeros((2, _B), dtype=np.float64)   # [map, sample]
    fallback_samples = []
    for c in range(_NCORES):
        sv = res.results[c]["stats_v"].astype(np.float64).sum(axis=0)  # [24]
        ss = res.results[c]["stats_s"].astype(np.float64).sum(axis=0)  # [8]
        for s in range(_SPC):
            b = c * _SPC + s
            for mi in range(2):
                col = (s * 2 + mi) * 3
                S_tot = sv[col]
                S_pos = sv[col + 1]
                if n_pos_mode == "act_sign":
                    n_pos_f = (ss[s * 2 + mi] + _N) / 2.0
                else:
                    n_pos_f = sv[col + 2]
                n_pos = int(round(n_pos_f))
                n_neg = _N - n_pos
                ok = abs(n_pos_f - n_pos) < 1e-3
                if ok and n_pos > 0 and (n_neg == 0 or 3 * n_pos >= n_neg):
                    pos_loss = S_pos / n_pos
                    neg_loss = (S_tot - S_pos) / n_neg if n_neg > 0 else 0.0
                    per_sample[mi, b] = pos_loss + neg_loss
                else:
                    fallback_samples.append((mi, b))

    if fallback_samples:
        for mi, b in fallback_samples:
            if mi == 0:
                pl = _host_pre_loss(rgt[b], rpred[b], m[b], _T_G)
                per_sample[mi, b] = _host_sample_loss(pl, rgt[b], _T_G)
            else:
                pl = _host_pre_loss(agt[b], apred[b], m[b], _T_A)
                per_sample[mi, b] = _host_sample_loss(pl, agt[b], _T_A)

    char_loss = per_sample[0].sum()
    affi_loss = per_sample[1].sum()
    out = _LAMBDA * char_loss / _B + affi_loss / _B
    return np.float32(out)


# revision 40
# speedup vs baseline: 1.0114x; 1.0114x over previous
"""Trainium2 Bass kernel for nn_MapLoss (topk_masking).

Strategy
--------
The reference loss needs, per sample and per map (region / affinity), only
three reductions:

    S_tot = sum(d^2 * mask)                 (d = clamped pred - gt)
    S_pos = sum((gt > t) * d^2 * mask)
    n_pos = #(gt > t)

because for the hard-negative top-k, k = min(3*n_pos, n_neg) and whenever
k == n_neg the "top-k sum of negatives" is just S_tot - S_pos (sum of all
negatives).  The rare general branches (3*n_pos < n_neg, or n_pos == 0)
are handled by an exact host fallback per sample (statistically never
taken for this input distribution).

Device plan (pure data parallel, 4 samples per core, no collectives):
  per sample-map tile [128, 2048] f32:
    1. custom DVE op CLAMPED_DIFF : d = (pred-gt) - (gt>t)*relu(pred-1)
    2. custom DVE op MASKED_SQ    : l = d^2*mask,  accum-> S_tot   (fused)
    3. scalar_tensor_tensor       : (gt>t)*l,      accum-> S_pos   (fused)
    4. ScalarE Sign activation    : sign(gt-t-eps), accum-> 2*n_pos - N
  Per-core output: per-partition stats tiles, final reduction on host.
"""

import os
import numpy as np
from contextlib import ExitStack

from concourse import bass, bacc, mybir
from concourse import tile
from concourse import bass_utils
import concourse.dve_ops as dve_ops_mod
from concourse.dve_ops import DveOp
from concourse.dve_spec import (
    Spec,
    Src0,
    Src1,
    C0,
    C1,
    Zero,
    relu,
    sq,
    lower,
    _has_src1,
)
from concourse.dve_uop import DveOpSpec
from operator import add as _op_add

# ---------------------------------------------------------------- constants
_B, _H, _W = 32, 512, 512
_N = _H * _W            # 262144 elements / sample
_P, _F = 128, 2048      # on-chip tile: 128 partitions x 2048 free  (= _N)
_NCORES = 8
_SPC = _B // _NCORES    # 4 samples per core
_T_G = 0.6              # THRESH_POSITIVE_REGION
_T_A = 0.65             # THRESH_POSITIVE_AFFINITY
_LAMBDA = 2.0
_TOPK_FALLBACK = 500

_IN_NAMES = ("rgt", "agt", "rpred", "apred", "m")

# ------------------------------------------------- custom DVE op definitions


def _register_dve_op(name, spec, subdim=False):
    """Register a custom DVE op in the process-local registry (additive;
    the documented extension point is appending to dve_ops.OPS)."""
    if name in dve_ops_mod._SUB_OPCODE_FOR_NAME:
        for op in dve_ops_mod.OPS:
            if op.name == name:
                return op
        raise RuntimeError(f"{name} in opcode map but not in OPS")
    row = max(dve_ops_mod._SUB_OPCODE_FOR_NAME.values()) + 1
    assert row < 0x20, "custom DVE opcode rows exhausted"
    shas = {}
    for ver in ("v3", "v4"):
        try:
            tmp = DveOpSpec(
                name=name, opcode=row, uops=lower(spec, ver=ver),
                rd1_en=_has_src1(spec),
            )
            shas[ver] = tmp.sha(ver)
        except Exception:
            pass
    assert "v3" in shas, f"{name}: failed to lower for TRN2"
    op = DveOp(name, spec, subdim, uops_sha=shas)
    dve_ops_mod.OPS.append(op)
    dve_ops_mod._SUB_OPCODE_FOR_NAME[name] = row
    dve_ops_mod.CUSTOM_DVE_SPECS[name] = spec
    return op


_OPS_CACHE = {}


def _get_custom_ops():
    if _OPS_CACHE:
        return _OPS_CACHE

    # d = (pred - gt) - (gt > t) * relu(pred - 1)
    clamped_diff = Spec(
        body=(Src0 - Src1) - (Src1 > C0) * relu(Src0 - C1),
        reference=lambda in0, in1, s0, s1, imm2: (
            (in0.astype(np.float32) - in1)
            - (in1 > s0) * np.maximum(in0.astype(np.float32) - s1, 0.0)
        ).astype(np.float32),
    )

    # l = in0^2 * in1 ; accum_out = sum(l)
    def _masked_sq_ref(in0, in1, s0, s1, imm2):
        b = (np.square(in0.astype(np.float32)) * in1).astype(np.float32)
        return b, b.reshape(b.shape[0], -1).sum(axis=-1, keepdims=True)

    masked_sq = Spec(
        body=sq(Src0) * Src1,
        accum=_op_add,
        accum_init=Zero,
        reference=_masked_sq_ref,
    )

    _OPS_CACHE["clamped_diff"] = _register_dve_op("ANT_MAPLOSS_CLAMPED_DIFF", clamped_diff)
    _OPS_CACHE["masked_sq"] = _register_dve_op("ANT_MAPLOSS_MASKED_SQ", masked_sq)
    return _OPS_CACHE


# ------------------------------------------------------------- bass builder

_NC_CACHE = {}


def _f32_exact(x):
    return float(np.float32(x))


def _build_bass(use_custom=True, repeats=1, loop=False, n_pos_mode="act_sign",
                dma="sync", frame=1, bufs_in=3, bufs_work=2, m_bufs=4):
    key = ("nc", use_custom, repeats, loop, n_pos_mode, dma, frame, bufs_in,
           bufs_work, m_bufs)
    if key in _NC_CACHE:
        return _NC_CACHE[key]
    ops = _get_custom_ops() if use_custom else None
    assert _SPC % frame == 0

    f32 = mybir.dt.float32
    bf16 = mybir.dt.bfloat16

    nc = bacc.Bacc(
        "TRN2", target_bir_lowering=False, debug=False, num_devices=_NCORES
    )
    ins = {
        name: nc.dram_tensor(
            name, [_SPC, _P, _F], f32, kind="ExternalInput"
        ).ap()
        for name in _IN_NAMES
    }
    # per sample-map stats: (S_tot, S_pos, n_pos) x 4 samples x 2 maps
    stats_v = nc.dram_tensor("stats_v", [_P, 24], f32, kind="ExternalOutput").ap()
    # scalar-engine stats (sign-sums = 2*n_pos - N), separate tile to avoid
    # cross-engine false deps on sv
    stats_s = nc.dram_tensor("stats_s", [_P, 8], f32, kind="ExternalOutput").ap()

    with tile.TileContext(nc) as tc, ExitStack() as ctx:
        inpool = ctx.enter_context(tc.tile_pool(name="in", bufs=bufs_in))
        # m is consumed by both maps (held ~2x longer than the others), give
        # it a deeper pool so the next samples' mask DMAs aren't stalled
        mpool = ctx.enter_context(tc.tile_pool(name="mp", bufs=m_bufs))
        workpool = ctx.enter_context(tc.tile_pool(name="work", bufs=bufs_work))
        statpool = ctx.enter_context(tc.tile_pool(name="stat", bufs=1))

        dma_eng = nc.sync if dma == "sync" else nc.gpsimd

        sv = statpool.tile([_P, 24], f32)
        ss = statpool.tile([_P, 8], f32)
        nc.scalar.memzero(ss[:])

        sign_bias = {}
        if n_pos_mode == "act_sign":
            # eps = 2^-24 shifts the threshold off the 2^-23 input grid so
            # sign() reproduces the strict '>' exactly (never hits 0).
            for thr in (_T_G, _T_A):
                bt = statpool.tile([_P, 1], f32, tag=f"bias{int(thr * 100)}")
                nc.gpsimd.memset(bt[:], -(_f32_exact(thr) + 2.0 ** -24))
                sign_bias[thr] = bt
        relu_bias = None
        if not use_custom:
            relu_bias = statpool.tile([_P, 1], f32, tag="relu_bias")
            nc.gpsimd.memset(relu_bias[:], -1.0)

        def emit_frame(f0):
            tiles = {}
            # order: region tensors + mask first so the region-map compute
            # can start after 3 transfers instead of 5
            for name in ("rgt", "rpred", "m", "agt", "apred"):
                pool = mpool if name == "m" else inpool
                t = pool.tile([_P, frame, _F], f32, tag=name)
                dma_eng.dma_start(
                    out=t[:],
                    in_=ins[name][f0 : f0 + frame].rearrange("s p f -> p s f"),
                )
                tiles[name] = t

            for j in range(frame):
                s = f0 + j
                for mi, (gtn, prn, thr) in enumerate(
                    (("rgt", "rpred", _T_G), ("agt", "apred", _T_A))
                ):
                    gt_t = tiles[gtn][:, j]
                    pr_t = tiles[prn][:, j]
                    m_t = tiles["m"][:, j]
                    thr32 = _f32_exact(thr)
                    col = (s * 2 + mi) * 3    # S_tot / S_pos / n_pos columns

                    d = workpool.tile([_P, _F], f32, tag="d")
                    if use_custom:
                        nc.vector._custom_dve(
                            ops["clamped_diff"], out=d[:], in0=pr_t,
                            in1=gt_t, s0=thr32, s1=1.0,
                        )
                    else:
                        # fallback chain built only from scalar_tensor_tensor
                        # and activation forms (same instruction structs as
                        # the custom path uses):
                        #   r = relu(pred-1); w = (gt>t)*r
                        #   t1 = (w*-1) + pred; d = (gt*-1) + t1
                        r = workpool.tile([_P, _F], f32, tag="r")
                        nc.scalar.activation(
                            r[:], pr_t, mybir.ActivationFunctionType.Relu,
                            bias=relu_bias[:], scale=1.0,
                        )
                        w = workpool.tile([_P, _F], f32, tag="w")
                        nc.vector.scalar_tensor_tensor(
                            out=w[:], in0=gt_t, scalar=thr32, in1=r[:],
                            op0=mybir.AluOpType.is_gt,
                            op1=mybir.AluOpType.mult,
                        )
                        t1 = workpool.tile([_P, _F], f32, tag="t1")
                        nc.vector.scalar_tensor_tensor(
                            out=t1[:], in0=w[:], scalar=-1.0, in1=pr_t,
                            op0=mybir.AluOpType.mult,
                            op1=mybir.AluOpType.add,
                        )
                        nc.vector.scalar_tensor_tensor(
                            out=d[:], in0=gt_t, scalar=-1.0, in1=t1[:],
                            op0=mybir.AluOpType.mult,
                            op1=mybir.AluOpType.add,
                        )

                    l = workpool.tile([_P, _F], f32, tag="l")
                    if use_custom:
                        nc.vector._custom_dve(
                            ops["masked_sq"], out=l[:], in0=d[:], in1=m_t,
                            accum_out=sv[:, col : col + 1],
                        )
                    else:
                        # dm = d*m ; l = d*dm with fused accum -> S_tot
                        dm = workpool.tile([_P, _F], f32, tag="dm")
                        nc.vector.scalar_tensor_tensor(
                            out=dm[:], in0=d[:], scalar=0.0, in1=m_t,
                            op0=mybir.AluOpType.bypass,
                            op1=mybir.AluOpType.mult,
                        )
                        nc.vector.scalar_tensor_tensor(
                            out=l[:], in0=d[:], scalar=0.0, in1=dm[:],
                            op0=mybir.AluOpType.bypass,
                            op1=mybir.AluOpType.mult,
                            accum_out=sv[:, col : col + 1],
                        )

                    z = workpool.tile([_P, 1], f32, tag="z")
                    nc.vector.scalar_tensor_tensor(
                        out=z.broadcast_to((_P, _F)), in0=gt_t, scalar=thr32,
                        in1=l[:],
                        op0=mybir.AluOpType.is_gt, op1=mybir.AluOpType.mult,
                        accum_out=sv[:, col + 1 : col + 2],
                    )

                    if n_pos_mode == "act_sign":
                        zs = workpool.tile([_P, _F], bf16, tag="zs")
                        nc.scalar.activation(
                            zs[:], gt_t, mybir.ActivationFunctionType.Sign,
                            bias=sign_bias[thr][:], scale=1.0,
                            accum_out=ss[:, s * 2 + mi : s * 2 + mi + 1],
                        )
                    else:
                        zp = workpool.tile([_P, 1], f32, tag="zp")
                        nc.vector.tensor_scalar(
                            out=zp.broadcast_to((_P, _F)), in0=gt_t,
                            scalar1=thr32,
                            scalar2=None, op0=mybir.AluOpType.is_gt,
                            op1=mybir.AluOpType.add,
                            accum_out=sv[:, col + 2 : col + 3],
                        )

        def emit_body():
            for f0 in range(0, _SPC, frame):
                emit_frame(f0)

        if loop and repeats > 1:
            with tc.For_i(0, repeats, 1):
                emit_body()
        else:
            for _ in range(repeats):
                emit_body()

        dma_eng.dma_start(out=stats_v[:], in_=sv[:])
        dma_eng.dma_start(out=stats_s[:], in_=ss[:])

    nc.compile()
    _NC_CACHE[key] = nc
    return nc


# ------------------------------------------------------------ host fallback


def _host_sample_loss(pre_loss, label, thresh):
    """Exact per-sample replica of reference._single_image_loss (one sample)."""
    pre_loss = pre_loss.astype(np.float64).ravel()
    label = label.astype(np.float32).ravel()
    pos_mask = label > np.float32(thresh)
    n_pos = int(pos_mask.sum())
    n_neg = pre_loss.size - n_pos
    if n_pos == 0:
        top = np.sort(pre_loss)[::-1][:_TOPK_FALLBACK]
        return float(top.mean())
    pos_loss = pre_loss[pos_mask].sum() / n_pos
    k = min(3 * n_pos, n_neg)
    if k <= 0:
        return float(pos_loss)
    neg_vals = np.sort(pre_loss[~pos_mask])[::-1]
    neg_loss = neg_vals[:k].sum() / k
    return float(pos_loss + neg_loss)


def _host_pre_loss(gt, pred, mask, thresh):
    gt = gt.astype(np.float32)
    pred = pred.astype(np.float32)
    clamped = np.where((gt > np.float32(thresh)) & (pred > np.float32(1.0)),
                       np.float32(1.0), pred)
    d = clamped.astype(np.float64) - gt.astype(np.float64)
    return d * d * mask.astype(np.float64)


# ------------------------------------------------------------------- bench


def _io_spec(nc):
    """Mirror run_bass_via_pjrt's input/output discovery."""
    partition_name = (
        nc.partition_id_tensor.name if nc.partition_id_tensor else None
    )
    in_names, out_names, out_avals, zero_outs = [], [], [], []
    import jax

    for alloc in nc.m.functions[0].allocations:
        if not isinstance(alloc, mybir.MemoryLocationSet):
            continue
        name = alloc.memorylocations[0].name
        if alloc.kind == "ExternalInput":
            if name != partition_name:
                in_names.append(name)
        elif alloc.kind == "ExternalOutput":
            out_names.append(name)
            shape = tuple(alloc.tensor_shape)
            dtype = mybir.dt.np(alloc.dtype)
            out_avals.append(jax.core.ShapedArray(shape, dtype))
            zero_outs.append(np.zeros(shape, dtype))
    return partition_name, in_names, out_names, out_avals, zero_outs


def _bench_one(inputs, iters=30, warmup=2, **build_kw):
    """Amortized per-execution wall time (ns) over `iters` queued runs."""
    import time
    import jax
    from jax.sharding import Mesh, PartitionSpec
    from jax.experimental.shard_map import shard_map
    from concourse import bass2jax
    from concourse.bass2jax import _bass_exec_p, install_neuronx_cc_hook

    install_neuronx_cc_hook()
    nc = _build_bass(**build_kw)
    pname, in_names, out_names, out_avals, zero_outs = _io_spec(nc)
    n_params, n_outs = len(in_names), len(out_names)
    all_names = in_names + out_names + ([pname] if pname else [])

    def _body(*args):
        operands = list(args)
        if pname is not None:
            operands.append(bass2jax.partition_id_tensor())
        outs = _bass_exec_p.bind(
            *operands,
            out_avals=tuple(out_avals),
            in_names=tuple(all_names),
            out_names=tuple(out_names),
            lowering_input_output_aliases=(),
            sim_require_finite=True,
            sim_require_nnan=True,
            nc=nc,
        )
        return tuple(outs)

    devices = jax.devices()[:_NCORES]
    mesh = Mesh(np.asarray(devices), ("core",))
    in_specs = (PartitionSpec("core"),) * (n_params + n_outs)
    out_specs = (PartitionSpec("core"),) * n_outs
    donate = tuple(range(n_params, n_params + n_outs))
    sharded = jax.jit(
        shard_map(_body, mesh=mesh, in_specs=in_specs, out_specs=out_specs,
                  check_rep=False),
        donate_argnums=donate, keep_unused=True,
    )

    arr = {k: np.ascontiguousarray(np.asarray(v, np.float32).reshape(_B, _P, _F))
           for k, v in inputs.items()}
    keyed = {"rgt": arr["region_score_gt"], "agt": arr["affinity_score_gt"],
             "rpred": arr["region_score_pred"], "apred": arr["affinity_score_pred"],
             "m": arr["mask"]}
    concat_in = [keyed[name] for name in in_names]  # [32,128,2048] = 8 cores x 4
    dev_in = [jax.device_put(a) for a in concat_in]

    def zeros():
        return [np.zeros((_NCORES * z.shape[0], *z.shape[1:]), z.dtype)
                for z in zero_outs]

    for _ in range(warmup):
        outs = sharded(*dev_in, *zeros())
        jax.block_until_ready(outs)
    zs = [zeros() for _ in range(iters)]
    t0 = time.perf_counter()
    results = [sharded(*dev_in, *z) for z in zs]
    jax.block_until_ready(results)
    t1 = time.perf_counter()
    return (t1 - t0) / iters * 1e9


def bench(inputs, rounds=3, k_lo=400, k_hi=1200, **build_kw):
    """Device time per kernel body (ns): slope between on-device For_i loops
    of k_lo and k_hi iterations.  K must be large enough that device time
    dominates the dispatch roundtrip, else async dispatch hides it."""
    est = []
    for _ in range(rounds):
        lo = _bench_one(inputs, iters=4, repeats=k_lo, loop=True, **build_kw)
        hi = _bench_one(inputs, iters=4, repeats=k_hi, loop=True, **build_kw)
        est.append((hi - lo) / (k_hi - k_lo))
    return float(np.median(est))


# ------------------------------------------------------------------- kernel

LAST_RESULTS = None


def kernel(**inputs):
    global LAST_RESULTS
    arr = {
        k: np.ascontiguousarray(np.asarray(v, dtype=np.float32))
        for k, v in inputs.items()
    }
    rgt = arr["region_score_gt"].reshape(_B, _P, _F)
    agt = arr["affinity_score_gt"].reshape(_B, _P, _F)
    rpred = arr["region_score_pred"].reshape(_B, _P, _F)
    apred = arr["affinity_score_pred"].reshape(_B, _P, _F)
    m = arr["mask"].reshape(_B, _P, _F)

    use_custom = os.environ.get("MAPLOSS_NO_CUSTOM", "0") != "1"
    n_pos_mode = os.environ.get("MAPLOSS_NPOS", "act_sign")
    dma = os.environ.get("MAPLOSS_DMA", "sync")
    nc = _build_bass(use_custom=use_custom, n_pos_mode=n_pos_mode, dma=dma)

    in_maps = []
    for c in range(_NCORES):
        sl = slice(c * _SPC, (c + 1) * _SPC)
        in_maps.append(
            {"rgt": rgt[sl], "agt": agt[sl], "rpred": rpred[sl],
             "apred": apred[sl], "m": m[sl]}
        )

    res = bass_utils.run_bass_kernel_spmd(
        nc, in_maps, core_ids=list(range(_NCORES))
    )
    LAST_RESULTS = res

    # ---- host-side finish (tiny): per-sample scalars ----------------------
    per_sample = np.zeros((2, _B), dtype=np.float64)   # [map, sample]
    fallback_samples = []
    for c in range(_NCORES):
        sv = res.results[c]["stats_v"].astype(np.float64).sum(axis=0)  # [24]
        ss = res.results[c]["stats_s"].astype(np.float64).sum(axis=0)  # [8]
        for s in range(_SPC):
            b = c * _SPC + s
            for mi in range(2):
                col = (s * 2 + mi) * 3
                S_tot = sv[col]
                S_pos = sv[col + 1]
                if n_pos_mode == "act_sign":
                    n_pos_f = (ss[s * 2 + mi] + _N) / 2.0
                else:
                    n_pos_f = sv[col + 2]
                n_pos = int(round(n_pos_f))
                n_neg = _N - n_pos
                ok = abs(n_pos_f - n_pos) < 1e-3
                if ok and n_pos > 0 and (n_neg == 0 or 3 * n_pos >= n_neg):
                    pos_loss = S_pos / n_pos
                    neg_loss = (S_tot - S_pos) / n_neg if n_neg > 0 else 0.0
                    per_sample[mi, b] = pos_loss + neg_loss
                else:
                    fallback_samples.append((mi, b))

    if fallback_samples:
        for mi, b in fallback_samples:
            if mi == 0:
                pl = _host_pre_loss(rgt[b], rpred[b], m[b], _T_G)
                per_sample[mi, b] = _host_sample_loss(pl, rgt[b], _T_G)
            else:
                pl = _host_pre_loss(agt[b], apred[b], m[b], _T_A)
                per_sample[mi, b] = _host_sample_loss(pl, agt[b], _T_A)

    char_loss = per_sample[0].sum()
    affi_loss = per_sample[1].sum()
    out = _LAMBDA * char_loss / _B + affi_loss / _B
    return np.float32(out)
